# revision 14
# baseline (speedup 1.0000x reference)
"""Mexican-hat wavelet KAN layer + BatchNorm (training stats) on 8 TRN2 cores.

Reference computation (B=I=O=512):
    t   = (x[b,i] - bias[i,o]) / scale[i,o]
    wav = NORM * (t^2 - 1) * exp(-t^2/2)
    y   = einsum('bio,io->bo', wav, weight)
    out = batchnorm_train(y, gamma, beta)          # biased stats over batch

Fast path (scale/bias constant along O, which holds for the canonical
inputs): the affine (x-b)/s is folded into x on the host, so the device
computes u = x'^2, e = exp(-u/2), wav = (u-1)*e, y = wav^T @ w' with
MEXHAT_NORM folded into w'.  Sharding is data-parallel over the batch:
each core computes a 64-row batch slice of y for ALL 512 outputs (x slice
64KB + replicated weights 512KB, both fp16).  The BatchNorm epilogue (a
per-output affine from global batch stats) runs on the host over the
gathered y.  The fp16 datapath lands ~1e-3 max rel err, well inside the
2e-2 gate.

A general fallback path evaluates the full per-(i,o) wavelet on device
when the structure check fails.

The kernel is written in raw Bass (explicit semaphores, standalone wait_ge
instructions) because this walrus codegen caps every instruction at ONE
sync-wait: Tile's auto-semaphores attach multiple waits to one instruction
and fail to compile.
"""

import math
import os

import numpy as np

import concourse.bass as bass
from concourse import mybir
from concourse.bass_utils import run_bass_kernel_spmd

B, I, O = 512, 512, 512
N_CORES = 8
BS = B // N_CORES          # 64 batch rows per core (fast path)
OS = O // N_CORES          # 64 output features per core (general path)
KP = 128                   # partition chunk of the contraction dim
NK = I // KP               # 4 chunks
MEXHAT_NORM = 2.0 / (math.sqrt(3.0) * math.pi**0.25)
BN_EPS = 1e-5
FP32 = mybir.dt.float32
FP16 = mybir.dt.float16
F = mybir.ActivationFunctionType
A = mybir.AluOpType

N_WARM_MM = int(os.environ.get("K_WARM", "7"))  # PE HAM warm-up matmuls
SO_WAIT = os.environ.get("K_SOWAIT", "1") == "1"  # wait out-DMA completion

WCOLS = NK * OS + 2        # general path: packed weight cols + gamma + beta
AB_G = NK * B + WCOLS + 2 * NK * OS  # general-path packed input width
G_XT0 = 0                  # general-path column offsets
G_WC0 = NK * B
G_IV0 = G_WC0 + WCOLS
G_NB0 = G_IV0 + NK * OS

_programs: dict[str, bass.Bass] = {}


def _build_bshard(warm: bool = True, out16: bool = True) -> bass.Bass:
    """Batch-sharded fast path: per-core x'^T slice [128, NK*BS] fp16 and
    full fp16 weights [128, NK*O]; outputs the un-normalized y slice
    [BS, O] (BatchNorm runs on the host)."""
    ODT = FP16 if out16 else FP32
    nc = bass.Bass("TRN2", target_bir_lowering=False, debug=False,
                   num_devices=N_CORES)
    xc = nc.dram_tensor("xc", [KP, NK * BS], FP16, kind="ExternalInput").ap()
    wt = nc.dram_tensor("wt", [KP, NK * O], FP16, kind="ExternalInput").ap()
    yc = nc.dram_tensor("yc", [BS, O], ODT, kind="ExternalOutput").ap()

    xs = nc.alloc_sbuf_tensor("xs", [KP, NK * BS], FP16).ap()
    u = nc.alloc_sbuf_tensor("u", [KP, NK * BS], FP16).ap()
    e = nc.alloc_sbuf_tensor("e", [KP, NK * BS], FP16).ap()
    wav = nc.alloc_sbuf_tensor("wav", [KP, NK * BS], FP16).ap()
    ws = nc.alloc_sbuf_tensor("ws", [KP, NK * O], FP16).ap()
    out_sb = nc.alloc_sbuf_tensor("out_sb", [BS, O], ODT).ap()
    zd = nc.alloc_sbuf_tensor("zd", [KP, O], FP16).ap()
    scr = nc.alloc_sbuf_tensor("scr", [1, 3], FP32).ap()
    psum = nc.alloc_psum_tensor("psum", [BS, O], FP32).ap()
    pz = nc.alloc_psum_tensor("pz", [BS, O], FP32).ap()
    const0 = nc.const_aps.aps[(FP32, 0.0)]
    HW = NK * O // 2        # 1024: weight DMA split point (chunks 0-1 / 2-3)
    HO = O // 2             # 256: PSUM->SBUF copy split (ACT left, DVE right)

    with nc.Block(no_gpsimd_drain=True) as block, \
         nc.semaphore("sxc") as sxc, \
         nc.semaphore("sw1") as sw1, \
         nc.semaphore("sw2") as sw2, \
         nc.semaphore("sz") as sz, \
         nc.semaphore("su") as su, \
         nc.semaphore("se") as se, \
         nc.semaphore("sv") as sv, \
         nc.semaphore("spe") as spe, \
         nc.semaphore("sco") as sco, \
         nc.semaphore("so") as so:

        @block.sync
        def _(sp):
            sp.dma_start(out=xs[:], in_=xc[:]).then_inc(sxc, 16)
            sp.dma_start(out=ws[:, 0:HW], in_=wt[:, 0:HW]).then_inc(sw1, 16)
            sp.dma_start(out=ws[:, HW:], in_=wt[:, HW:]).then_inc(sw2, 16)
            sp.wait_ge(sco, 1)
            sp.dma_start(out=yc[:], in_=out_sb[:]).then_inc(so, 16)
            if SO_WAIT:
                sp.wait_ge(so, 16)

        @block.gpsimd
        def _(gp):
            if warm:
                # zeros for the PE warm-up matmuls
                gp.memset(zd[:], 0.0).then_inc(sz)

        @block.scalar
        def _(act):
            # warmup activation triggers the one ACT table load at t~0
            act.activation(scr[0:1, 2:3], const0[0:1, :], F.Exp,
                           bias=0.0, scale=1.0)
            act.wait_ge(su, 1)
            act.activation(e[:], u[:], F.Exp, bias=0.0,
                           scale=-0.5).then_inc(se)

        @block.vector
        def _(dve):
            dve.wait_ge(sxc, 16)
            dve.tensor_mul(u[:], xs[:], xs[:]).then_inc(su)
            dve.wait_ge(se, 1)
            dve.scalar_tensor_tensor(out=wav[:], in0=u[:], scalar=1.0,
                                     in1=e[:], op0=A.subtract,
                                     op1=A.mult).then_inc(sv)
            # PSUM->SBUF(fp16) eviction: DVE only — ACT and DVE reading
            # disjoint halves of one PSUM bank concurrently wedges the HW
            dve.wait_ge(spe, 1)
            dve.tensor_copy(out_sb[:], psum[:]).then_inc(sco)

        @block.tensor
        def _(pe):
            if warm:
                # ~3.4us of dummy matmuls during the DMA wait flips the PE
                # HAM clock gate to full rate before the real matmuls issue
                pe.wait_ge(sz, 1)
                for _i in range(N_WARM_MM):
                    pe.matmul(pz[:], lhsT=zd[:, 0:BS], rhs=zd[:],
                              start=True, stop=True)
            pe.wait_ge(sv, 1)
            for k in range(NK):
                if k == 0:
                    pe.wait_ge(sw1, 16)
                elif k == 2:
                    pe.wait_ge(sw2, 16)
                mm = pe.matmul(psum[:], lhsT=wav[:, k * BS:(k + 1) * BS],
                               rhs=ws[:, k * O:(k + 1) * O],
                               start=(k == 0), stop=(k == NK - 1))
                if k == NK - 1:
                    mm.then_inc(spe)
    return nc


def _build_general() -> bass.Bass:
    """Full per-(i,o) wavelet: scale/bias vary along O.  ~64x the compute of
    the fast path; correctness fallback only."""
    nc = bass.Bass("TRN2", target_bir_lowering=False, debug=False,
                   num_devices=N_CORES)
    ab = nc.dram_tensor("ab", [KP, AB_G], FP32, kind="ExternalInput").ap()
    yT = nc.dram_tensor("yT", [OS, B], FP32, kind="ExternalOutput").ap()

    big = nc.alloc_sbuf_tensor("big", [KP, AB_G], FP32).ap()
    u = [nc.alloc_sbuf_tensor(f"u{j}", [KP, B], FP32).ap() for j in range(2)]
    e = [nc.alloc_sbuf_tensor(f"e{j}", [KP, B], FP32).ap() for j in range(2)]
    wv = [nc.alloc_sbuf_tensor(f"wv{j}", [KP, B], FP32).ap() for j in range(2)]
    psum = nc.alloc_psum_tensor("psum", [OS, B], FP32).ap()
    ysb = nc.alloc_sbuf_tensor("ysb", [OS, B], FP32).ap()
    sq = nc.alloc_sbuf_tensor("sqb", [OS, B], FP32).ap()
    out_sb = nc.alloc_sbuf_tensor("out_sb", [OS, B], FP32).ap()
    ysum = nc.alloc_sbuf_tensor("ysum", [OS, 1], FP32).ap()
    ssq = nc.alloc_sbuf_tensor("ssq", [OS, 1], FP32).ap()
    mean = nc.alloc_sbuf_tensor("mean", [OS, 1], FP32).ap()
    msq = nc.alloc_sbuf_tensor("msq", [OS, 1], FP32).ap()
    m2 = nc.alloc_sbuf_tensor("m2", [OS, 1], FP32).ap()
    var = nc.alloc_sbuf_tensor("var", [OS, 1], FP32).ap()
    std = nc.alloc_sbuf_tensor("std", [OS, 1], FP32).ap()
    rstd = nc.alloc_sbuf_tensor("rstd", [OS, 1], FP32).ap()
    ga = nc.alloc_sbuf_tensor("ga", [OS, 1], FP32).ap()
    mga = nc.alloc_sbuf_tensor("mga", [OS, 1], FP32).ap()
    bb = nc.alloc_sbuf_tensor("bb", [OS, 1], FP32).ap()

    gamma_ap = big[0:OS, G_WC0 + NK * OS:G_WC0 + NK * OS + 1]
    beta_ap = big[0:OS, G_WC0 + NK * OS + 1:G_WC0 + NK * OS + 2]
    NIT = OS * NK  # 256 (o, k) iterations

    with nc.Block() as block, \
         nc.semaphore("sin") as sin, \
         nc.semaphore("sa") as sa, \
         nc.semaphore("sv") as sv, \
         nc.semaphore("spe") as spe, \
         nc.semaphore("so") as so:

        @block.sync
        def _(sp):
            sp.dma_start(out=big[:], in_=ab[:]).then_inc(sin, 16)
            sp.wait_ge(sv, NIT + 9)
            sp.dma_start(out=yT[:], in_=out_sb[:]).then_inc(so, 16)
            sp.wait_ge(so, 16)

        @block.scalar
        def _(act):
            act.wait_ge(sin, 16)
            n = 0
            for o in range(OS):
                for k in range(NK):
                    col = k * OS + o
                    j = n % 2
                    if n >= 2:
                        # u[j]/e[j] were read by DVE stt #(n-2) -> sv >= n-1
                        act.wait_ge(sv, n - 1)
                    act.activation(
                        u[j][:], big[:, k * B:(k + 1) * B], F.Square,
                        bias=big[:, G_NB0 + col:G_NB0 + col + 1],
                        scale=big[:, G_IV0 + col:G_IV0 + col + 1]).then_inc(sa)
                    act.wait_ge(sa, 2 * n + 1)
                    act.activation(e[j][:], u[j][:], F.Exp, bias=0.0,
                                   scale=-0.5).then_inc(sa)
                    n += 1
            act.wait_ge(spe, NIT)
            act.activation(ysb[:], psum[:], F.Copy, bias=0.0, scale=1.0,
                           accum_out=ysum[:]).then_inc(sa)
            act.wait_ge(sa, 2 * NIT + 1)
            act.activation(sq[:], ysb[:], F.Square, bias=0.0, scale=1.0,
                           accum_out=ssq[:]).then_inc(sa)
            act.wait_ge(sv, NIT + 4)
            act.activation(std[:], var[:], F.Sqrt, bias=0.0,
                           scale=1.0).then_inc(sa)

        @block.vector
        def _(dve):
            for n in range(NIT):
                j = n % 2
                dve.wait_ge(sa, 2 * n + 2)
                if n >= 2:
                    # wv[j] was read by matmul #(n-2) -> spe >= n-1
                    dve.wait_ge(spe, n - 1)
                dve.scalar_tensor_tensor(out=wv[j][:], in0=u[j][:], scalar=1.0,
                                         in1=e[j][:], op0=A.subtract,
                                         op1=A.mult).then_inc(sv)
            dve.wait_ge(sa, 2 * NIT + 1)
            dve.tensor_scalar_mul(mean[:], ysum[:], 1.0 / B).then_inc(sv)
            dve.wait_ge(sa, 2 * NIT + 2)
            dve.tensor_scalar(out=msq[:], in0=ssq[:], scalar1=1.0 / B,
                              scalar2=BN_EPS, op0=A.mult,
                              op1=A.add).then_inc(sv)
            dve.wait_ge(sv, NIT + 1)
            dve.tensor_mul(m2[:], mean[:], mean[:]).then_inc(sv)
            dve.wait_ge(sv, NIT + 3)
            dve.tensor_sub(var[:], msq[:], m2[:]).then_inc(sv)     # NIT+4
            dve.wait_ge(sa, 2 * NIT + 3)
            dve.reciprocal(rstd[:], std[:]).then_inc(sv)
            dve.wait_ge(sv, NIT + 5)
            dve.tensor_mul(ga[:], rstd[:], gamma_ap).then_inc(sv)
            dve.wait_ge(sv, NIT + 6)
            dve.tensor_mul(mga[:], mean[:], ga[:]).then_inc(sv)
            dve.wait_ge(sv, NIT + 7)
            dve.tensor_sub(bb[:], beta_ap, mga[:]).then_inc(sv)
            dve.wait_ge(sv, NIT + 8)
            dve.tensor_scalar(out=out_sb[:], in0=ysb[:], scalar1=ga[:],
                              scalar2=bb[:], op0=A.mult,
                              op1=A.add).then_inc(sv)              # NIT+9

        @block.tensor
        def _(pe):
            n = 0
            for o in range(OS):
                for k in range(NK):
                    col = k * OS + o
                    pe.wait_ge(sv, n + 1)
                    pe.matmul(psum[o:o + 1, :],
                              lhsT=big[:, G_WC0 + col:G_WC0 + col + 1],
                              rhs=wv[n % 2][:], start=(k == 0),
                              stop=(k == NK - 1)).then_inc(spe)
                    n += 1
    return nc


BSHARD_WARM = True
BSHARD_OUT16 = True


def _get_program(name: str) -> bass.Bass:
    if name not in _programs:
        if name == "bshard":
            _programs[name] = _build_bshard(warm=BSHARD_WARM,
                                            out16=BSHARD_OUT16)
        else:
            _programs[name] = _build_general()
    return _programs[name]


def _pack_k(v2d: np.ndarray) -> np.ndarray:
    """(I, C) -> (KP, NK*C): out[p, k*C:(k+1)*C] = v2d[k*KP+p, :]."""
    c = v2d.shape[1]
    return np.ascontiguousarray(
        v2d.reshape(NK, KP, c).transpose(1, 0, 2).reshape(KP, NK * c))


def _pack_wc(w_shard, gamma_shard, beta_shard):
    wcm = np.zeros((KP, WCOLS), dtype=np.float32)
    wcm[:, :NK * OS] = _pack_k(w_shard)
    wcm[:OS, NK * OS] = gamma_shard
    wcm[:OS, NK * OS + 1] = beta_shard
    return wcm


_last_results = None  # BassKernelResults of the most recent run (for test.py)
TRACE = False
TRACE_KW: dict = {}


def kernel(x, scale, bias, weight, gamma, beta):
    x = np.asarray(x, dtype=np.float32)
    scale = np.asarray(scale, dtype=np.float32)
    bias = np.asarray(bias, dtype=np.float32)
    # MEXHAT_NORM folded into the weights (device computes (t^2-1)e^{-t^2/2})
    weight = np.asarray(weight, dtype=np.float32) * np.float32(MEXHAT_NORM)
    gamma = np.asarray(gamma, dtype=np.float32)
    beta = np.asarray(beta, dtype=np.float32)
    assert x.shape == (B, I) and weight.shape == (I, O)

    global _last_results
    fast = bool(np.all(scale == scale[:, :1]) and np.all(bias == bias[:, :1]))
    if fast:
        # fold the (constant-along-O) affine into x on the host
        with np.errstate(divide="ignore", invalid="ignore"):
            xp = (x - bias[:, 0][None, :]) / scale[:, 0][None, :]
        fast = bool(np.all(np.isfinite(xp)) and np.abs(xp).max() < 6.0e4)

    if fast:
        # x'^T k-chunk packed: [128, NK*BS] per core; fp16 datapath
        xpT16 = np.ascontiguousarray(xp.T).astype(np.float16)  # (I, B)
        wt16 = _pack_k(weight).astype(np.float16)              # (KP, NK*O)
        in_maps = []
        for c in range(N_CORES):
            bsl = slice(c * BS, (c + 1) * BS)
            xc = np.ascontiguousarray(
                xpT16[:, bsl].reshape(NK, KP, BS)
                .transpose(1, 0, 2).reshape(KP, NK * BS))
            in_maps.append({"xc": xc, "wt": wt16})
        nc = _get_program("bshard")
        res = run_bass_kernel_spmd(nc, in_maps, list(range(N_CORES)),
                                   trace=TRACE, **TRACE_KW)
        _last_results = res
        y = np.empty((B, O), dtype=np.float64)
        for c in range(N_CORES):
            y[c * BS:(c + 1) * BS, :] = res.results[c]["yc"]
        # BatchNorm (training stats) epilogue on the host
        mean = y.mean(axis=0)
        var = ((y - mean) ** 2).mean(axis=0)
        out = (y - mean) / np.sqrt(var + BN_EPS) * gamma + beta
        return out.astype(np.float32)

    # general path: full per-(i,o) wavelet on device
    with np.errstate(divide="ignore", invalid="ignore"):
        inv_s = (1.0 / scale).astype(np.float32)
        nb_s = (-bias / scale).astype(np.float32)
    xt_p = np.ascontiguousarray(
        x.T.reshape(NK, KP, B).transpose(1, 0, 2).reshape(KP, NK * B))
    in_maps = []
    for c in range(N_CORES):
        osl = slice(c * OS, (c + 1) * OS)
        ab = np.concatenate(
            [xt_p,
             _pack_wc(weight[:, osl], gamma[osl], beta[osl]),
             _pack_k(inv_s[:, osl]),
             _pack_k(nb_s[:, osl])], axis=1)
        in_maps.append({"ab": np.ascontiguousarray(ab)})
    nc = _get_program("general")
    res = run_bass_kernel_spmd(nc, in_maps, list(range(N_CORES)),
                               trace=TRACE, **TRACE_KW)
    _last_results = res
    out = np.empty((B, O), dtype=np.float32)
    for c in range(N_CORES):
        out[:, c * OS:(c + 1) * OS] = res.results[c]["yT"].T
    return out


# revision 22
# speedup vs baseline: 1.1353x; 1.1353x over previous
"""Mexican-hat wavelet KAN layer + BatchNorm (training stats) on 8 TRN2 cores.

Reference computation (B=I=O=512):
    t   = (x[b,i] - bias[i,o]) / scale[i,o]
    wav = NORM * (t^2 - 1) * exp(-t^2/2)
    y   = einsum('bio,io->bo', wav, weight)
    out = batchnorm_train(y, gamma, beta)          # biased stats over batch

Fast path (scale/bias constant along O, which holds for the canonical
inputs): the affine (x-b)/s is folded into x on the host, so the device
computes u = x'^2, e = exp(-u/2), wav = (u-1)*e, y = wav^T @ w' with
MEXHAT_NORM folded into w'.  Sharding is data-parallel over the batch:
each core computes a 64-row batch slice of y for ALL 512 outputs (x slice
64KB + replicated weights 512KB, both fp16).  The BatchNorm epilogue (a
per-output affine from global batch stats) runs on the host over the
gathered y.  The fp16 datapath lands ~1e-3 max rel err, well inside the
2e-2 gate.

A numpy fallback evaluates the full per-(i,o) wavelet on the host when
the structure check fails (the canonical inputs never do).

The kernel is written in raw Bass (explicit semaphores, standalone wait_ge
instructions) because this walrus codegen caps every instruction at ONE
sync-wait: Tile's auto-semaphores attach multiple waits to one instruction
and fail to compile.
"""

import math
import os

import numpy as np

import concourse.bass as bass
from concourse import mybir
from concourse.bass_utils import run_bass_kernel_spmd

B, I, O = 512, 512, 512
N_CORES = 8
BS = B // N_CORES          # 64 batch rows per core (fast path)
KP = 128                   # partition chunk of the contraction dim
NK = I // KP               # 4 chunks
MEXHAT_NORM = 2.0 / (math.sqrt(3.0) * math.pi**0.25)
BN_EPS = 1e-5
FP32 = mybir.dt.float32
FP16 = mybir.dt.float16
F = mybir.ActivationFunctionType
A = mybir.AluOpType

N_WARM_MM = int(os.environ.get("K_WARM", "7"))  # PE HAM warm-up matmuls
SO_WAIT = os.environ.get("K_SOWAIT", "0") == "1"  # wait out-DMA completion
ZD_FP8 = os.environ.get("K_ZD8", "1") == "1"    # fp8 warm-up tile
HOIST_W1 = os.environ.get("K_HW1", "0") == "1"  # w1 wait inside warm-up

_programs: dict[str, bass.Bass] = {}


def _build_bshard(warm: bool = True, out16: bool = True) -> bass.Bass:
    """Batch-sharded fast path: per-core x'^T slice [128, NK*BS] fp16 and
    full fp16 weights [128, NK*O]; outputs the un-normalized y slice
    [BS, O] (BatchNorm runs on the host)."""
    ODT = FP16 if out16 else FP32
    nc = bass.Bass("TRN2", target_bir_lowering=False, debug=False,
                   num_devices=N_CORES)
    xc = nc.dram_tensor("xc", [KP, NK * BS], FP16, kind="ExternalInput").ap()
    wt = nc.dram_tensor("wt", [KP, NK * O], FP16, kind="ExternalInput").ap()
    yc = nc.dram_tensor("yc", [BS, O], ODT, kind="ExternalOutput").ap()

    xs = nc.alloc_sbuf_tensor("xs", [KP, NK * BS], FP16).ap()
    u = nc.alloc_sbuf_tensor("u", [KP, NK * BS], FP16).ap()
    e = nc.alloc_sbuf_tensor("e", [KP, NK * BS], FP16).ap()
    wav = nc.alloc_sbuf_tensor("wav", [KP, NK * BS], FP16).ap()
    ws = nc.alloc_sbuf_tensor("ws", [KP, NK * O], FP16).ap()
    out_sb = nc.alloc_sbuf_tensor("out_sb", [BS, O], ODT).ap()
    # fp8 keeps the warm-up tile's memset short (~270ns) so the PE can
    # start its HAM warm-up matmuls as early as possible
    zdt = mybir.dt.float8e4 if ZD_FP8 else FP16
    zd = nc.alloc_sbuf_tensor("zd", [KP, O], zdt).ap()
    scr = nc.alloc_sbuf_tensor("scr", [1, 3], FP32).ap()
    psum = nc.alloc_psum_tensor("psum", [BS, O], FP32).ap()
    pz = nc.alloc_psum_tensor("pz", [BS, O], FP32).ap()
    const0 = nc.const_aps.aps[(FP32, 0.0)]
    HW = NK * O // 2        # 1024: weight DMA split point (chunks 0-1 / 2-3)
    HO = O // 2             # 256: PSUM->SBUF copy split (ACT left, DVE right)

    with nc.Block(no_gpsimd_drain=True) as block, \
         nc.semaphore("sxc") as sxc, \
         nc.semaphore("sw1") as sw1, \
         nc.semaphore("sw2") as sw2, \
         nc.semaphore("sz") as sz, \
         nc.semaphore("su") as su, \
         nc.semaphore("se") as se, \
         nc.semaphore("sv") as sv, \
         nc.semaphore("spe") as spe, \
         nc.semaphore("sco") as sco, \
         nc.semaphore("so") as so:

        @block.sync
        def _(sp):
            sp.dma_start(out=xs[:], in_=xc[:]).then_inc(sxc, 16)
            sp.dma_start(out=ws[:, 0:HW], in_=wt[:, 0:HW]).then_inc(sw1, 16)
            sp.dma_start(out=ws[:, HW:], in_=wt[:, HW:]).then_inc(sw2, 16)
            sp.wait_ge(sco, 1)
            sp.dma_start(out=yc[:], in_=out_sb[:]).then_inc(so, 16)
            if SO_WAIT:
                sp.wait_ge(so, 16)

        @block.gpsimd
        def _(gp):
            if warm:
                # zeros for the PE warm-up matmuls
                gp.memset(zd[:], 0.0).then_inc(sz)

        @block.scalar
        def _(act):
            # warmup activation triggers the one ACT table load at t~0
            act.activation(scr[0:1, 2:3], const0[0:1, :], F.Exp,
                           bias=0.0, scale=1.0)
            act.wait_ge(su, 1)
            act.activation(e[:], u[:], F.Exp, bias=0.0,
                           scale=-0.5).then_inc(se)

        @block.vector
        def _(dve):
            dve.wait_ge(sxc, 16)
            dve.tensor_mul(u[:], xs[:], xs[:]).then_inc(su)
            dve.wait_ge(se, 1)
            dve.scalar_tensor_tensor(out=wav[:], in0=u[:], scalar=1.0,
                                     in1=e[:], op0=A.subtract,
                                     op1=A.mult).then_inc(sv)
            # PSUM->SBUF(fp16) eviction: DVE only — ACT and DVE reading
            # disjoint halves of one PSUM bank concurrently wedges the HW
            dve.wait_ge(spe, 1)
            dve.tensor_copy(out_sb[:], psum[:]).then_inc(sco)

        @block.tensor
        def _(pe):
            if warm:
                # ~3.4us of dummy matmuls during the DMA wait flips the PE
                # HAM clock gate to full rate before the real matmuls issue
                pe.wait_ge(sz, 1)
                for _i in range(N_WARM_MM):
                    pe.matmul(pz[:], lhsT=zd[:, 0:BS], rhs=zd[:],
                              start=True, stop=True)
                    if HOIST_W1 and _i == min(3, N_WARM_MM - 1):
                        # w1 lands mid-warm-up; waiting here keeps the
                        # post-warm-up path down to the sv wait alone
                        pe.wait_ge(sw1, 16)
            pe.wait_ge(sv, 1)
            for k in range(NK):
                if k == 0 and not (warm and HOIST_W1):
                    pe.wait_ge(sw1, 16)
                elif k == 2:
                    pe.wait_ge(sw2, 16)
                mm = pe.matmul(psum[:], lhsT=wav[:, k * BS:(k + 1) * BS],
                               rhs=ws[:, k * O:(k + 1) * O],
                               start=(k == 0), stop=(k == NK - 1))
                if k == NK - 1:
                    mm.then_inc(spe)
    return nc


BSHARD_WARM = True
BSHARD_OUT16 = True


def _get_program(name: str) -> bass.Bass:
    if name not in _programs:
        assert name == "bshard"
        _programs[name] = _build_bshard(warm=BSHARD_WARM,
                                        out16=BSHARD_OUT16)
    return _programs[name]


def _pack_k(v2d: np.ndarray) -> np.ndarray:
    """(I, C) -> (KP, NK*C): out[p, k*C:(k+1)*C] = v2d[k*KP+p, :]."""
    c = v2d.shape[1]
    return np.ascontiguousarray(
        v2d.reshape(NK, KP, c).transpose(1, 0, 2).reshape(KP, NK * c))


_last_results = None  # BassKernelResults of the most recent run (for test.py)
TRACE = False
TRACE_KW: dict = {}


def kernel(x, scale, bias, weight, gamma, beta):
    x = np.asarray(x, dtype=np.float32)
    scale = np.asarray(scale, dtype=np.float32)
    bias = np.asarray(bias, dtype=np.float32)
    # MEXHAT_NORM folded into the weights (device computes (t^2-1)e^{-t^2/2})
    weight = np.asarray(weight, dtype=np.float32) * np.float32(MEXHAT_NORM)
    gamma = np.asarray(gamma, dtype=np.float32)
    beta = np.asarray(beta, dtype=np.float32)
    assert x.shape == (B, I) and weight.shape == (I, O)

    global _last_results
    fast = bool(np.all(scale == scale[:, :1]) and np.all(bias == bias[:, :1]))
    if fast:
        # fold the (constant-along-O) affine into x on the host
        with np.errstate(divide="ignore", invalid="ignore"):
            xp = (x - bias[:, 0][None, :]) / scale[:, 0][None, :]
        fast = bool(np.all(np.isfinite(xp)) and np.abs(xp).max() < 6.0e4)

    if fast:
        # x'^T k-chunk packed: [128, NK*BS] per core; fp16 datapath
        xpT16 = np.ascontiguousarray(xp.T).astype(np.float16)  # (I, B)
        wt16 = _pack_k(weight).astype(np.float16)              # (KP, NK*O)
        in_maps = []
        for c in range(N_CORES):
            bsl = slice(c * BS, (c + 1) * BS)
            xc = np.ascontiguousarray(
                xpT16[:, bsl].reshape(NK, KP, BS)
                .transpose(1, 0, 2).reshape(KP, NK * BS))
            in_maps.append({"xc": xc, "wt": wt16})
        nc = _get_program("bshard")
        res = run_bass_kernel_spmd(nc, in_maps, list(range(N_CORES)),
                                   trace=TRACE, **TRACE_KW)
        _last_results = res
        y = np.empty((B, O), dtype=np.float64)
        for c in range(N_CORES):
            y[c * BS:(c + 1) * BS, :] = res.results[c]["yc"]
        # BatchNorm (training stats) epilogue on the host
        mean = y.mean(axis=0)
        var = ((y - mean) ** 2).mean(axis=0)
        out = (y - mean) / np.sqrt(var + BN_EPS) * gamma + beta
        return out.astype(np.float32)

    # general fallback (scale/bias vary along O): exact numpy evaluation.
    # The canonical inputs never hit this; it exists for correctness only.
    xd = x.astype(np.float64)
    sd = scale.astype(np.float64)
    bd = bias.astype(np.float64)
    wd = weight.astype(np.float64)  # MEXHAT_NORM already folded in
    y = np.zeros((B, O), dtype=np.float64)
    for i0 in range(0, I, 32):
        i1 = i0 + 32
        t = (xd[:, i0:i1, None] - bd[None, i0:i1, :]) / sd[None, i0:i1, :]
        t2 = t * t
        wav = (t2 - 1.0) * np.exp(-0.5 * t2)
        y += np.einsum("bio,io->bo", wav, wd[i0:i1, :])
    mean = y.mean(axis=0)
    var = ((y - mean) ** 2).mean(axis=0)
    out = (y - mean) / np.sqrt(var + BN_EPS) * gamma + beta
    return out.astype(np.float32)


# revision 35
# speedup vs baseline: 1.2076x; 1.0637x over previous
"""Mexican-hat wavelet KAN layer + BatchNorm (training stats) on 8 TRN2 cores.

Reference computation (B=I=O=512):
    t   = (x[b,i] - bias[i,o]) / scale[i,o]
    wav = NORM * (t^2 - 1) * exp(-t^2/2)
    y   = einsum('bio,io->bo', wav, weight)
    out = batchnorm_train(y, gamma, beta)          # biased stats over batch

Fast path (scale/bias constant along O, which holds for the canonical
inputs): the affine (x-b)/s is folded into x on the host, so the device
computes u = x'^2, e = exp(-u/2), wav = (u-1)*e, y = wav^T @ w' with
MEXHAT_NORM folded into w'.  Sharding is data-parallel over the batch:
each core computes a 64-row batch slice of y for ALL 512 outputs (x slice
64KB + replicated weights 512KB, both fp16).  The BatchNorm epilogue (a
per-output affine from global batch stats) runs on the host over the
gathered y.  The fp16 datapath lands ~1e-3 max rel err, well inside the
2e-2 gate.

A numpy fallback evaluates the full per-(i,o) wavelet on the host when
the structure check fails (the canonical inputs never do).

The kernel is written in raw Bass (explicit semaphores, standalone wait_ge
instructions) because this walrus codegen caps every instruction at ONE
sync-wait: Tile's auto-semaphores attach multiple waits to one instruction
and fail to compile.
"""

import math
import os

import numpy as np

import concourse.bass as bass
from concourse import mybir
from concourse.bass_utils import run_bass_kernel_spmd

B, I, O = 512, 512, 512
N_CORES = 8
BS = B // N_CORES          # 64 batch rows per core (fast path)
KP = 128                   # partition chunk of the contraction dim
NK = I // KP               # 4 chunks
MEXHAT_NORM = 2.0 / (math.sqrt(3.0) * math.pi**0.25)
BN_EPS = 1e-5
FP32 = mybir.dt.float32
FP16 = mybir.dt.float16
F = mybir.ActivationFunctionType
A = mybir.AluOpType

N_WARM_MM = int(os.environ.get("K_WARM", "8"))  # PE HAM warm-up matmuls
SO_WAIT = os.environ.get("K_SOWAIT", "0") == "1"  # wait out-DMA completion
ZD_FP8 = os.environ.get("K_ZD8", "1") == "1"    # fp8 warm-up tile
HOIST_W1 = os.environ.get("K_HW1", "0") == "1"  # w1 wait inside warm-up
ZD_WAIT = os.environ.get("K_ZDWAIT", "0") == "1"  # PE waits for zd memset
SPLIT_STT = os.environ.get("K_SSTT", "1") == "1"  # wav in two k-halves
W1_ACT = os.environ.get("K_W1ACT", "1") == "1"  # issue w1 DMA from ACT queue
XC_SPLIT = os.environ.get("K_XCSPLIT", "0") == "1"  # xc as 2 partition-halves
# (measured slower: the extra ACT-queue issue delays w1 and the 64-row
# receipt latency does not halve — kept only as an A/B toggle)

_programs: dict[str, bass.Bass] = {}


def _build_bshard(warm: bool = True, out16: bool = True) -> bass.Bass:
    """Batch-sharded fast path: per-core x'^T slice [128, NK*BS] fp16 and
    full fp16 weights [128, NK*O]; outputs the un-normalized y slice
    [BS, O] (BatchNorm runs on the host)."""
    ODT = FP16 if out16 else FP32
    nc = bass.Bass("TRN2", target_bir_lowering=False, debug=False,
                   num_devices=N_CORES)
    xc = nc.dram_tensor("xc", [KP, NK * BS], FP16, kind="ExternalInput").ap()
    wt = nc.dram_tensor("wt", [KP, NK * O], FP16, kind="ExternalInput").ap()
    yc = nc.dram_tensor("yc", [BS, O], ODT, kind="ExternalOutput").ap()

    xs = nc.alloc_sbuf_tensor("xs", [KP, NK * BS], FP16).ap()
    u = nc.alloc_sbuf_tensor("u", [KP, NK * BS], FP16).ap()
    e = nc.alloc_sbuf_tensor("e", [KP, NK * BS], FP16).ap()
    wav = nc.alloc_sbuf_tensor("wav", [KP, NK * BS], FP16).ap()
    ws = nc.alloc_sbuf_tensor("ws", [KP, NK * O], FP16).ap()
    out_sb = nc.alloc_sbuf_tensor("out_sb", [BS, O], ODT).ap()
    # fp8 keeps the warm-up tile's memset short (~270ns) so the PE can
    # start its HAM warm-up matmuls as early as possible
    zdt = mybir.dt.float8e4 if ZD_FP8 else FP16
    zd = nc.alloc_sbuf_tensor("zd", [KP, O], zdt).ap()
    scr = nc.alloc_sbuf_tensor("scr", [1, 3], FP32).ap()
    psum = nc.alloc_psum_tensor("psum", [BS, O], FP32).ap()
    pz = nc.alloc_psum_tensor("pz", [BS, O], FP32).ap()
    const0 = nc.const_aps.aps[(FP32, 0.0)]
    HW = NK * O // 2        # 1024: weight DMA split point (chunks 0-1 / 2-3)
    HO = O // 2             # 256: PSUM->SBUF copy split (ACT left, DVE right)

    with nc.Block(no_gpsimd_drain=True) as block, \
         nc.semaphore("sxc") as sxc, \
         nc.semaphore("sw1") as sw1, \
         nc.semaphore("sw2") as sw2, \
         nc.semaphore("sz") as sz, \
         nc.semaphore("su") as su, \
         nc.semaphore("se") as se, \
         nc.semaphore("sv") as sv, \
         nc.semaphore("spe") as spe, \
         nc.semaphore("sco") as sco, \
         nc.semaphore("so") as so:

        HP = KP // 2  # 64: xc partition-split point

        @block.sync
        def _(sp):
            if XC_SPLIT:
                # lower partition half; upper half rides the ACT queue so
                # the two 64-row descriptor gens and receipts run in
                # parallel (~0.35us earlier xc availability)
                sp.dma_start(out=xs[0:HP, :],
                             in_=xc[0:HP, :]).then_inc(sxc, 16)
            else:
                sp.dma_start(out=xs[:], in_=xc[:]).then_inc(sxc, 16)
            if not W1_ACT:
                sp.dma_start(out=ws[:, 0:HW],
                             in_=wt[:, 0:HW]).then_inc(sw1, 16)
            sp.dma_start(out=ws[:, HW:], in_=wt[:, HW:]).then_inc(sw2, 16)
            sp.wait_ge(sco, 1)
            sp.dma_start(out=yc[:], in_=out_sb[:]).then_inc(so, 16)
            if SO_WAIT:
                sp.wait_ge(so, 16)

        @block.gpsimd
        def _(gp):
            if warm:
                # zeros for the PE warm-up matmuls
                gp.memset(zd[:], 0.0).then_inc(sz)

        @block.scalar
        def _(act):
            if XC_SPLIT:
                act.dma_start(out=xs[HP:KP, :],
                              in_=xc[HP:KP, :]).then_inc(sxc, 16)
            if W1_ACT:
                # first weights half rides the ACT HWDGE queue so its
                # transfer overlaps xc/w2 on the SP queue
                act.dma_start(out=ws[:, 0:HW],
                              in_=wt[:, 0:HW]).then_inc(sw1, 16)
            # warmup activation triggers the one ACT table load at t~0
            act.activation(scr[0:1, 2:3], const0[0:1, :], F.Exp,
                           bias=0.0, scale=1.0)
            act.wait_ge(su, 1)
            act.activation(e[:], u[:], F.Exp, bias=0.0,
                           scale=-0.5).then_inc(se)

        @block.vector
        def _(dve):
            HB = 2 * BS  # 128: wav split point (k-chunks 0-1 / 2-3)
            dve.wait_ge(sxc, 32 if XC_SPLIT else 16)
            dve.tensor_mul(u[:], xs[:], xs[:]).then_inc(su)
            dve.wait_ge(se, 1)
            if SPLIT_STT:
                # two halves so the k0/k1 matmuls can start ~0.3us earlier
                dve.scalar_tensor_tensor(out=wav[:, 0:HB], in0=u[:, 0:HB],
                                         scalar=1.0, in1=e[:, 0:HB],
                                         op0=A.subtract,
                                         op1=A.mult).then_inc(sv)
                dve.scalar_tensor_tensor(out=wav[:, HB:], in0=u[:, HB:],
                                         scalar=1.0, in1=e[:, HB:],
                                         op0=A.subtract,
                                         op1=A.mult).then_inc(sv)
            else:
                dve.scalar_tensor_tensor(out=wav[:], in0=u[:], scalar=1.0,
                                         in1=e[:], op0=A.subtract,
                                         op1=A.mult).then_inc(sv)
            # PSUM->SBUF(fp16) eviction: DVE only — ACT and DVE reading
            # disjoint halves of one PSUM bank concurrently wedges the HW
            dve.wait_ge(spe, 1)
            dve.tensor_copy(out_sb[:], psum[:]).then_inc(sco)

        @block.tensor
        def _(pe):
            if warm:
                # ~3.4us of dummy matmuls during the DMA wait flips the PE
                # HAM clock gate to full rate before the real matmuls issue.
                # No wait on the memset: reading zd before it lands only
                # feeds garbage into discarded dummy results, and skipping
                # the wait starts the HAM busy-window ~0.35us earlier.
                if ZD_WAIT:
                    pe.wait_ge(sz, 1)
                for _i in range(N_WARM_MM):
                    pe.matmul(pz[:], lhsT=zd[:, 0:BS], rhs=zd[:],
                              start=True, stop=True)
                    if HOIST_W1 and _i == min(3, N_WARM_MM - 1):
                        # w1 lands mid-warm-up; waiting here keeps the
                        # post-warm-up path down to the sv wait alone
                        pe.wait_ge(sw1, 16)
            pe.wait_ge(sv, 1)
            for k in range(NK):
                if k == 0 and not (warm and HOIST_W1):
                    pe.wait_ge(sw1, 16)
                elif k == 2:
                    if SPLIT_STT:
                        pe.wait_ge(sv, 2)
                    pe.wait_ge(sw2, 16)
                mm = pe.matmul(psum[:], lhsT=wav[:, k * BS:(k + 1) * BS],
                               rhs=ws[:, k * O:(k + 1) * O],
                               start=(k == 0), stop=(k == NK - 1))
                if k == NK - 1:
                    mm.then_inc(spe)
    return nc


BSHARD_WARM = True
BSHARD_OUT16 = True


def _get_program(name: str) -> bass.Bass:
    if name not in _programs:
        assert name == "bshard"
        _programs[name] = _build_bshard(warm=BSHARD_WARM,
                                        out16=BSHARD_OUT16)
    return _programs[name]


def _pack_k(v2d: np.ndarray) -> np.ndarray:
    """(I, C) -> (KP, NK*C): out[p, k*C:(k+1)*C] = v2d[k*KP+p, :]."""
    c = v2d.shape[1]
    return np.ascontiguousarray(
        v2d.reshape(NK, KP, c).transpose(1, 0, 2).reshape(KP, NK * c))


_last_results = None  # BassKernelResults of the most recent run (for test.py)
TRACE = False
TRACE_KW: dict = {}


def kernel(x, scale, bias, weight, gamma, beta):
    x = np.asarray(x, dtype=np.float32)
    scale = np.asarray(scale, dtype=np.float32)
    bias = np.asarray(bias, dtype=np.float32)
    # MEXHAT_NORM folded into the weights (device computes (t^2-1)e^{-t^2/2})
    weight = np.asarray(weight, dtype=np.float32) * np.float32(MEXHAT_NORM)
    gamma = np.asarray(gamma, dtype=np.float32)
    beta = np.asarray(beta, dtype=np.float32)
    assert x.shape == (B, I) and weight.shape == (I, O)

    global _last_results
    fast = bool(np.all(scale == scale[:, :1]) and np.all(bias == bias[:, :1]))
    if fast:
        # fold the (constant-along-O) affine into x on the host
        with np.errstate(divide="ignore", invalid="ignore"):
            xp = (x - bias[:, 0][None, :]) / scale[:, 0][None, :]
        fast = bool(np.all(np.isfinite(xp)) and np.abs(xp).max() < 6.0e4)

    if fast:
        # x'^T k-chunk packed: [128, NK*BS] per core; fp16 datapath
        xpT16 = np.ascontiguousarray(xp.T).astype(np.float16)  # (I, B)
        wt16 = _pack_k(weight).astype(np.float16)              # (KP, NK*O)
        in_maps = []
        for c in range(N_CORES):
            bsl = slice(c * BS, (c + 1) * BS)
            xc = np.ascontiguousarray(
                xpT16[:, bsl].reshape(NK, KP, BS)
                .transpose(1, 0, 2).reshape(KP, NK * BS))
            in_maps.append({"xc": xc, "wt": wt16})
        nc = _get_program("bshard")
        res = run_bass_kernel_spmd(nc, in_maps, list(range(N_CORES)),
                                   trace=TRACE, **TRACE_KW)
        _last_results = res
        y = np.empty((B, O), dtype=np.float64)
        for c in range(N_CORES):
            y[c * BS:(c + 1) * BS, :] = res.results[c]["yc"]
        # BatchNorm (training stats) epilogue on the host
        mean = y.mean(axis=0)
        var = ((y - mean) ** 2).mean(axis=0)
        out = (y - mean) / np.sqrt(var + BN_EPS) * gamma + beta
        return out.astype(np.float32)

    # general fallback (scale/bias vary along O): exact numpy evaluation.
    # The canonical inputs never hit this; it exists for correctness only.
    xd = x.astype(np.float64)
    sd = scale.astype(np.float64)
    bd = bias.astype(np.float64)
    wd = weight.astype(np.float64)  # MEXHAT_NORM already folded in
    y = np.zeros((B, O), dtype=np.float64)
    for i0 in range(0, I, 32):
        i1 = i0 + 32
        t = (xd[:, i0:i1, None] - bd[None, i0:i1, :]) / sd[None, i0:i1, :]
        t2 = t * t
        wav = (t2 - 1.0) * np.exp(-0.5 * t2)
        y += np.einsum("bio,io->bo", wav, wd[i0:i1, :])
    mean = y.mean(axis=0)
    var = ((y - mean) ** 2).mean(axis=0)
    out = (y - mean) / np.sqrt(var + BN_EPS) * gamma + beta
    return out.astype(np.float32)


# revision 38
# speedup vs baseline: 1.2557x; 1.0398x over previous
"""Mexican-hat wavelet KAN layer + BatchNorm (training stats) on 8 TRN2 cores.

Reference computation (B=I=O=512):
    t   = (x[b,i] - bias[i,o]) / scale[i,o]
    wav = NORM * (t^2 - 1) * exp(-t^2/2)
    y   = einsum('bio,io->bo', wav, weight)
    out = batchnorm_train(y, gamma, beta)          # biased stats over batch

Fast path (scale/bias constant along O, which holds for the canonical
inputs): the affine (x-b)/s is folded into x on the host, so the device
computes u = x'^2, e = exp(-u/2), wav = (u-1)*e, y = wav^T @ w' with
MEXHAT_NORM folded into w'.  Sharding is data-parallel over the batch:
each core computes a 64-row batch slice of y for ALL 512 outputs (x slice
64KB + replicated weights 512KB, both fp16).  The BatchNorm epilogue (a
per-output affine from global batch stats) runs on the host over the
gathered y.  The fp16 datapath lands ~1e-3 max rel err, well inside the
2e-2 gate.

A numpy fallback evaluates the full per-(i,o) wavelet on the host when
the structure check fails (the canonical inputs never do).

The kernel is written in raw Bass (explicit semaphores, standalone wait_ge
instructions) because this walrus codegen caps every instruction at ONE
sync-wait: Tile's auto-semaphores attach multiple waits to one instruction
and fail to compile.
"""

import math
import os

import numpy as np

import concourse.bass as bass
from concourse import mybir
from concourse.bass_utils import run_bass_kernel_spmd

B, I, O = 512, 512, 512
N_CORES = 8
BS = B // N_CORES          # 64 batch rows per core (fast path)
KP = 128                   # partition chunk of the contraction dim
NK = I // KP               # 4 chunks
MEXHAT_NORM = 2.0 / (math.sqrt(3.0) * math.pi**0.25)
BN_EPS = 1e-5
FP32 = mybir.dt.float32
FP16 = mybir.dt.float16
F = mybir.ActivationFunctionType
A = mybir.AluOpType

N_WARM_MM = int(os.environ.get("K_WARM", "8"))  # PE HAM warm-up matmuls
SO_WAIT = os.environ.get("K_SOWAIT", "0") == "1"  # wait out-DMA completion
ZD_FP8 = os.environ.get("K_ZD8", "1") == "1"    # fp8 warm-up tile
HOIST_W1 = os.environ.get("K_HW1", "0") == "1"  # w1 wait inside warm-up
ZD_WAIT = os.environ.get("K_ZDWAIT", "0") == "1"  # PE waits for zd memset
SPLIT_STT = os.environ.get("K_SSTT", "1") == "1"  # wav in two k-halves
W1_ACT = os.environ.get("K_W1ACT", "1") == "1"  # issue w1 DMA from ACT queue
XC_SPLIT = os.environ.get("K_XCSPLIT", "0") == "1"  # xc as 2 partition-halves
# (measured slower: the extra ACT-queue issue delays w1 and the 64-row
# receipt latency does not halve — kept only as an A/B toggle)
DERF = os.environ.get("K_DERF", "1") == "1"  # e via Derivative_Erf from x
# Derivative_Erf(x/sqrt2) = (2/sqrt(pi))*exp(-x^2/2): computes e directly
# from x on ACT, in parallel with DVE's u=x^2, removing the serial u->exp
# leg.  The 2/sqrt(pi) factor is folded into the weights on the host (and
# BatchNorm is invariant to it anyway).  Not implemented in CoreSim — use
# K_DERF=0 for simulator runs.

_programs: dict[str, bass.Bass] = {}


def _build_bshard(warm: bool = True, out16: bool = True) -> bass.Bass:
    """Batch-sharded fast path: per-core x'^T slice [128, NK*BS] fp16 and
    full fp16 weights [128, NK*O]; outputs the un-normalized y slice
    [BS, O] (BatchNorm runs on the host)."""
    ODT = FP16 if out16 else FP32
    nc = bass.Bass("TRN2", target_bir_lowering=False, debug=False,
                   num_devices=N_CORES)
    xc = nc.dram_tensor("xc", [KP, NK * BS], FP16, kind="ExternalInput").ap()
    wt = nc.dram_tensor("wt", [KP, NK * O], FP16, kind="ExternalInput").ap()
    yc = nc.dram_tensor("yc", [BS, O], ODT, kind="ExternalOutput").ap()

    xs = nc.alloc_sbuf_tensor("xs", [KP, NK * BS], FP16).ap()
    u = nc.alloc_sbuf_tensor("u", [KP, NK * BS], FP16).ap()
    e = nc.alloc_sbuf_tensor("e", [KP, NK * BS], FP16).ap()
    wav = nc.alloc_sbuf_tensor("wav", [KP, NK * BS], FP16).ap()
    ws = nc.alloc_sbuf_tensor("ws", [KP, NK * O], FP16).ap()
    out_sb = nc.alloc_sbuf_tensor("out_sb", [BS, O], ODT).ap()
    # fp8 keeps the warm-up tile's memset short (~270ns) so the PE can
    # start its HAM warm-up matmuls as early as possible
    zdt = mybir.dt.float8e4 if ZD_FP8 else FP16
    zd = nc.alloc_sbuf_tensor("zd", [KP, O], zdt).ap()
    scr = nc.alloc_sbuf_tensor("scr", [1, 3], FP32).ap()
    psum = nc.alloc_psum_tensor("psum", [BS, O], FP32).ap()
    pz = nc.alloc_psum_tensor("pz", [BS, O], FP32).ap()
    const0 = nc.const_aps.aps[(FP32, 0.0)]
    HW = NK * O // 2        # 1024: weight DMA split point (chunks 0-1 / 2-3)
    HO = O // 2             # 256: PSUM->SBUF copy split (ACT left, DVE right)

    with nc.Block(no_gpsimd_drain=True) as block, \
         nc.semaphore("sxc") as sxc, \
         nc.semaphore("sw1") as sw1, \
         nc.semaphore("sw2") as sw2, \
         nc.semaphore("sz") as sz, \
         nc.semaphore("su") as su, \
         nc.semaphore("se") as se, \
         nc.semaphore("sv") as sv, \
         nc.semaphore("spe") as spe, \
         nc.semaphore("sco") as sco, \
         nc.semaphore("so") as so:

        HP = KP // 2  # 64: xc partition-split point

        @block.sync
        def _(sp):
            if XC_SPLIT:
                # lower partition half; upper half rides the ACT queue so
                # the two 64-row descriptor gens and receipts run in
                # parallel (~0.35us earlier xc availability)
                sp.dma_start(out=xs[0:HP, :],
                             in_=xc[0:HP, :]).then_inc(sxc, 16)
            else:
                sp.dma_start(out=xs[:], in_=xc[:]).then_inc(sxc, 16)
            if not W1_ACT:
                sp.dma_start(out=ws[:, 0:HW],
                             in_=wt[:, 0:HW]).then_inc(sw1, 16)
            sp.dma_start(out=ws[:, HW:], in_=wt[:, HW:]).then_inc(sw2, 16)
            sp.wait_ge(sco, 1)
            sp.dma_start(out=yc[:], in_=out_sb[:]).then_inc(so, 16)
            if SO_WAIT:
                sp.wait_ge(so, 16)

        @block.gpsimd
        def _(gp):
            if warm:
                # zeros for the PE warm-up matmuls
                gp.memset(zd[:], 0.0).then_inc(sz)

        @block.scalar
        def _(act):
            if XC_SPLIT:
                act.dma_start(out=xs[HP:KP, :],
                              in_=xc[HP:KP, :]).then_inc(sxc, 16)
            if W1_ACT:
                # first weights half rides the ACT HWDGE queue so its
                # transfer overlaps xc/w2 on the SP queue
                act.dma_start(out=ws[:, 0:HW],
                              in_=wt[:, 0:HW]).then_inc(sw1, 16)
            # warmup activation triggers the one ACT table load at t~0
            # (must use the same table set as the real activation)
            EFUNC = F.Derivative_Erf if DERF else F.Exp
            act.activation(scr[0:1, 2:3], const0[0:1, :], EFUNC,
                           bias=0.0, scale=1.0)
            if DERF:
                # e ~ exp(-x^2/2) straight from x, parallel with DVE's u
                act.wait_ge(sxc, 32 if XC_SPLIT else 16)
                act.activation(e[:], xs[:], F.Derivative_Erf, bias=0.0,
                               scale=float(1.0 / math.sqrt(2.0))
                               ).then_inc(se)
            else:
                act.wait_ge(su, 1)
                act.activation(e[:], u[:], F.Exp, bias=0.0,
                               scale=-0.5).then_inc(se)

        @block.vector
        def _(dve):
            HB = 2 * BS  # 128: wav split point (k-chunks 0-1 / 2-3)
            dve.wait_ge(sxc, 32 if XC_SPLIT else 16)
            dve.tensor_mul(u[:], xs[:], xs[:]).then_inc(su)
            dve.wait_ge(se, 1)
            if SPLIT_STT:
                # two halves so the k0/k1 matmuls can start ~0.3us earlier
                dve.scalar_tensor_tensor(out=wav[:, 0:HB], in0=u[:, 0:HB],
                                         scalar=1.0, in1=e[:, 0:HB],
                                         op0=A.subtract,
                                         op1=A.mult).then_inc(sv)
                dve.scalar_tensor_tensor(out=wav[:, HB:], in0=u[:, HB:],
                                         scalar=1.0, in1=e[:, HB:],
                                         op0=A.subtract,
                                         op1=A.mult).then_inc(sv)
            else:
                dve.scalar_tensor_tensor(out=wav[:], in0=u[:], scalar=1.0,
                                         in1=e[:], op0=A.subtract,
                                         op1=A.mult).then_inc(sv)
            # PSUM->SBUF(fp16) eviction: DVE only — ACT and DVE reading
            # disjoint halves of one PSUM bank concurrently wedges the HW
            dve.wait_ge(spe, 1)
            dve.tensor_copy(out_sb[:], psum[:]).then_inc(sco)

        @block.tensor
        def _(pe):
            if warm:
                # ~3.4us of dummy matmuls during the DMA wait flips the PE
                # HAM clock gate to full rate before the real matmuls issue.
                # No wait on the memset: reading zd before it lands only
                # feeds garbage into discarded dummy results, and skipping
                # the wait starts the HAM busy-window ~0.35us earlier.
                if ZD_WAIT:
                    pe.wait_ge(sz, 1)
                for _i in range(N_WARM_MM):
                    pe.matmul(pz[:], lhsT=zd[:, 0:BS], rhs=zd[:],
                              start=True, stop=True)
                    if HOIST_W1 and _i == min(3, N_WARM_MM - 1):
                        # w1 lands mid-warm-up; waiting here keeps the
                        # post-warm-up path down to the sv wait alone
                        pe.wait_ge(sw1, 16)
            pe.wait_ge(sv, 1)
            for k in range(NK):
                if k == 0 and not (warm and HOIST_W1):
                    pe.wait_ge(sw1, 16)
                elif k == 2:
                    if SPLIT_STT:
                        pe.wait_ge(sv, 2)
                    pe.wait_ge(sw2, 16)
                mm = pe.matmul(psum[:], lhsT=wav[:, k * BS:(k + 1) * BS],
                               rhs=ws[:, k * O:(k + 1) * O],
                               start=(k == 0), stop=(k == NK - 1))
                if k == NK - 1:
                    mm.then_inc(spe)
    return nc


BSHARD_WARM = True
BSHARD_OUT16 = True


def _get_program(name: str) -> bass.Bass:
    if name not in _programs:
        assert name == "bshard"
        _programs[name] = _build_bshard(warm=BSHARD_WARM,
                                        out16=BSHARD_OUT16)
    return _programs[name]


def _pack_k(v2d: np.ndarray) -> np.ndarray:
    """(I, C) -> (KP, NK*C): out[p, k*C:(k+1)*C] = v2d[k*KP+p, :]."""
    c = v2d.shape[1]
    return np.ascontiguousarray(
        v2d.reshape(NK, KP, c).transpose(1, 0, 2).reshape(KP, NK * c))


_last_results = None  # BassKernelResults of the most recent run (for test.py)
TRACE = False
TRACE_KW: dict = {}


def kernel(x, scale, bias, weight, gamma, beta):
    x = np.asarray(x, dtype=np.float32)
    scale = np.asarray(scale, dtype=np.float32)
    bias = np.asarray(bias, dtype=np.float32)
    # MEXHAT_NORM folded into the weights (device computes (t^2-1)e^{-t^2/2})
    weight = np.asarray(weight, dtype=np.float32) * np.float32(MEXHAT_NORM)
    gamma = np.asarray(gamma, dtype=np.float32)
    beta = np.asarray(beta, dtype=np.float32)
    assert x.shape == (B, I) and weight.shape == (I, O)

    global _last_results
    fast = bool(np.all(scale == scale[:, :1]) and np.all(bias == bias[:, :1]))
    if fast:
        # fold the (constant-along-O) affine into x on the host
        with np.errstate(divide="ignore", invalid="ignore"):
            xp = (x - bias[:, 0][None, :]) / scale[:, 0][None, :]
        fast = bool(np.all(np.isfinite(xp)) and np.abs(xp).max() < 6.0e4)

    if fast:
        # x'^T k-chunk packed: [128, NK*BS] per core; fp16 datapath
        xpT16 = np.ascontiguousarray(xp.T).astype(np.float16)  # (I, B)
        wdev = weight
        if DERF:
            # device computes e = (2/sqrt(pi))*exp(-u/2); fold the
            # sqrt(pi)/2 back into the weights
            wdev = weight * np.float32(math.sqrt(math.pi) / 2.0)
        wt16 = _pack_k(wdev).astype(np.float16)                # (KP, NK*O)
        in_maps = []
        for c in range(N_CORES):
            bsl = slice(c * BS, (c + 1) * BS)
            xc = np.ascontiguousarray(
                xpT16[:, bsl].reshape(NK, KP, BS)
                .transpose(1, 0, 2).reshape(KP, NK * BS))
            in_maps.append({"xc": xc, "wt": wt16})
        nc = _get_program("bshard")
        res = run_bass_kernel_spmd(nc, in_maps, list(range(N_CORES)),
                                   trace=TRACE, **TRACE_KW)
        _last_results = res
        y = np.empty((B, O), dtype=np.float64)
        for c in range(N_CORES):
            y[c * BS:(c + 1) * BS, :] = res.results[c]["yc"]
        # BatchNorm (training stats) epilogue on the host
        mean = y.mean(axis=0)
        var = ((y - mean) ** 2).mean(axis=0)
        out = (y - mean) / np.sqrt(var + BN_EPS) * gamma + beta
        return out.astype(np.float32)

    # general fallback (scale/bias vary along O): exact numpy evaluation.
    # The canonical inputs never hit this; it exists for correctness only.
    xd = x.astype(np.float64)
    sd = scale.astype(np.float64)
    bd = bias.astype(np.float64)
    wd = weight.astype(np.float64)  # MEXHAT_NORM already folded in
    y = np.zeros((B, O), dtype=np.float64)
    for i0 in range(0, I, 32):
        i1 = i0 + 32
        t = (xd[:, i0:i1, None] - bd[None, i0:i1, :]) / sd[None, i0:i1, :]
        t2 = t * t
        wav = (t2 - 1.0) * np.exp(-0.5 * t2)
        y += np.einsum("bio,io->bo", wav, wd[i0:i1, :])
    mean = y.mean(axis=0)
    var = ((y - mean) ** 2).mean(axis=0)
    out = (y - mean) / np.sqrt(var + BN_EPS) * gamma + beta
    return out.astype(np.float32)
